# revision 43
# baseline (speedup 1.0000x reference)
"""CAM-style self-attention kernel for Trainium2 (8 NeuronCores, SPMD).

Reference computation (per batch sample b):
    q = x[b].reshape(N, C)                 # N = H*W = 4096, C = 512
    E = q @ q.T                            # [N, N]
    A = softmax(rowmax(E) - E, axis=-1)    # == exp(rowmin(E) - E) / rowsum
    out = A @ q
    y[b] = alpha * out + x[b]

Sharding: data-parallel over batch B=8 -> one sample per NeuronCore.

V2 design (vs fp16 baseline):
- E-phase matmuls stay fp16 (argmin needs ~1e-2 E precision), but the PSUM
  readback negates E into a fp16 SBUF panel so the row-min becomes a row-MAX
  (gpsimd only has max ops), computed as a 3-level pairwise tree on the idle
  Pool engine + a small DVE tail.
- Attention weights exp to fp8e4 directly (ACT, per-row bias). PE transposes
  the fp8 panel (walrus requires output element step 2; odd bytes are zero),
  and the psum->sbuf readback copies the step-2 pairs as bitcast fp16 at DVE
  2x rate.
- O-phase runs fp8 DoubleRow matmuls (0.5 cyc/row, K=256/MM): out = T8 @ q8h
  + T8 @ q8l accumulated in one PSUM bank, where q = q8h + q8l splits into
  two fp8 casts (quantization error ~0.2%). The softmax normalizer Z comes
  free from ones-column DoubleRow matmuls against the same weights, so the
  quantized attention self-normalizes (convex combination is exact).
- x is re-streamed from DRAM for the final residual add (frees 8MB q32).
"""

import os
import numpy as np

import concourse.bass as bass
import concourse.mybir as mybir
import concourse.tile as tile
from concourse.bass_utils import run_bass_kernel_spmd
from concourse.masks import make_identity

B, H, W, C = 8, 64, 64, 512
N = H * W            # 4096
P = 128              # partitions
NT = N // P          # 32 row bands
KC = C // P          # 4 contraction chunks for E (K = C = 512)
CH = 512             # free-dim chunk (one PSUM bank of fp32)
NCH = N // CH        # 8 chunks per row band
NPAIR = NT // 2      # 16 DoubleRow v-band pairs

F32 = mybir.dt.float32
F16 = mybir.dt.float16
F8 = mybir.dt.float8e4
DR = mybir.MatmulPerfMode.DoubleRow

_CACHE = {}
LAST_RESULTS = None  # stashed BassKernelResults for test harness introspection


def _build_bass():
    nc = bass.Bass()
    x_d = nc.declare_dram_parameter("x", [N, C], F32, isOutput=False)
    a_d = nc.declare_dram_parameter("alpha", [1, 1], F32, isOutput=False)
    y_d = nc.declare_dram_parameter("y", [N, C], F32, isOutput=True)

    with tile.TileContext(nc) as tc:
        with (
            tc.tile_pool(name="persist", bufs=1) as persist,
            tc.tile_pool(name="qscr", bufs=2) as qscr,
            tc.tile_pool(name="ework", bufs=4) as ework,
            tc.tile_pool(name="t8p", bufs=2) as t8p,
            tc.tile_pool(name="t8tp", bufs=3) as t8tp,
            tc.tile_pool(name="treep", bufs=1) as treep,
            tc.tile_pool(name="stats", bufs=6) as stats,
            tc.tile_pool(name="xres", bufs=3) as xres,
            tc.tile_pool(name="outp", bufs=2) as outp,
            tc.tile_pool(name="psum", bufs=2, space="PSUM") as psum,
            tc.tile_pool(name="dram", bufs=1, space="DRAM") as dram,
        ):
            # DRAM scratch for the symmetric-E round-trip: scratch[v][u] holds
            # E-block (u, v) [128, 128] fp16 row-major (+80-shifted), written
            # by band u's panel DMA, read back (per-block transposed) as band
            # v's lower panel.
            scratch = dram.tile([NT, 28, P, P], F16)
            # ---- persistent tiles ----
            qT16 = persist.tile([P, KC, N], F16)     # qT16[p, k, n] = q[n, k*128+p]
            q8h = persist.tile([P, NPAIR, 2, C], F8)  # q8h[p, j, i, c] ~ q[(2j+i)*128+p, c]
            q8l = persist.tile([P, NPAIR, 2, C], F8)  # fp8 remainder: q ~ q8h + q8l
            ident32 = persist.tile([P, P], F32)
            make_identity(nc, ident32)
            ident16 = persist.tile([P, P], F16)
            nc.vector.tensor_copy(ident16, ident32)
            ident8 = persist.tile([P, P], F8)
            nc.vector.tensor_copy(ident8, ident32)
            ones8 = persist.tile([P, 2, 16], F8)     # DR pair dim needs stride%16==0
            nc.vector.memset(ones8, 1.0)
            alpha_sb = persist.tile([P, 1], F32)
            # E-shift: store e16 = E + 80 so the row-min region (~-81) sits
            # near 0 where fp16 ulp is ~0.004 instead of 0.0625. The shift
            # cancels exactly in exp(rmin_shifted - e16).
            c80 = persist.tile([P, 1], F32)
            nc.vector.memset(c80, 80.0)

            # broadcast-load alpha across all partitions
            a_ap = a_d[:, :]
            a_bc = bass.AP(tensor=a_ap.tensor, offset=a_ap.offset,
                           ap=[[0, P], [1, 1]])
            nc.gpsimd.dma_start(out=alpha_sb, in_=a_bc)

            # Warm-up PE op so later transposes carry a single sync wait.
            warm_ps = psum.tile([P, 8, P, 2], F8, tag="pt",
                                padded_shape=[P, 8, P, 2])
            nc.tensor.transpose(warm_ps[:, 0, :, 0], ident8, ident8)
            warm_sb = stats.tile([P, 1], F16, tag="warm")
            nc.vector.tensor_copy(warm_sb, warm_ps[:, 0, 0:1, 0])

            saved = {}

            def e_chunk(e16, i, j, jmin=0, hold=None):
                """E chunk j of band i -> +80-shifted fp16 panel columns.
                Chunks pair up in a 2-bank psum tile; each full pair is read
                back by one 1024-wide copy (DVE/ACT alternating); a trailing
                odd chunk gets a 512-wide copy."""
                l = j - jmin
                if l % 2 == 0:
                    hold[0] = psum.tile([P, 2, CH], F32, tag="e", bufs=2, name="ep")
                ep = hold[0]
                for k in range(KC):
                    nc.tensor.matmul(
                        ep[:, l % 2, :],
                        qT16[:, k, i * P:(i + 1) * P],
                        qT16[:, k, j * CH:(j + 1) * CH],
                        start=(k == 0),
                        stop=(k == KC - 1),
                    )
                last = (j == NCH - 1)
                if l % 2 == 1 or last:
                    n = (l % 2) + 1
                    dst = e16[:, (j - n + 1) * CH:(j + 1) * CH]
                    if (l // 2) % 2 == 0:
                        nc.vector.tensor_scalar_add(
                            dst.rearrange("p (a c) -> p a c", a=n),
                            ep[:, :n, :], 80.0)
                    else:
                        nc.scalar.activation(
                            dst.rearrange("p (a c) -> p a c", a=n),
                            ep[:, :n, :],
                            mybir.ActivationFunctionType.Copy,
                            bias=80.0, scale=1.0)

            def e_finish(e16, i):
                """Row min via DVE pairwise fp16 tree (2x mode) + reduce."""
                t1 = treep.tile([P, 2048], F16, tag="t1")
                nc.vector.tensor_tensor(t1, e16[:, :2048], e16[:, 2048:],
                                        op=mybir.AluOpType.min)
                t2 = treep.tile([P, 1024], F16, tag="t2")
                nc.vector.tensor_tensor(t2, t1[:, :1024], t1[:, 1024:],
                                        op=mybir.AluOpType.min)
                t3 = treep.tile([P, 512], F16, tag="t3")
                nc.vector.tensor_tensor(t3, t2[:, :512], t2[:, 512:],
                                        op=mybir.AluOpType.min)
                rmin = stats.tile([P, 1], F16, tag="rmin")
                nc.vector.tensor_reduce(rmin, t3, axis=mybir.AxisListType.X,
                                        op=mybir.AluOpType.min)
                bias = stats.tile([P, 1], F32, tag="bias")
                nc.vector.tensor_copy(bias, rmin)
                saved[i] = (e16, bias)

            # ---- preamble: load x, build qT16 (fp16 transposes) and the fp8
            #      DR-pair layouts; band-0 E chunks interleave as qT columns
            #      become ready ----
            e16_0 = ework.tile([P, N], F16, tag="e16")
            e16_1 = ework.tile([P, N], F16, tag="e16")
            hold0, hold1 = [None], [None]
            for g in range(8):  # 8 groups x 4 bands
                q32g = qscr.tile([P, 4, C], F32, tag="q32")
                nc.sync.dma_start(
                    out=q32g,
                    in_=x_d[g * 512:(g + 1) * 512, :].rearrange(
                        "(t p) c -> p t c", p=P),
                )
                q16g = qscr.tile([P, 4, C], F16, tag="q16")
                nc.vector.tensor_copy(q16g, q32g)
                # fp8 hi: strided write into the DR pair layout
                h_slice = q8h[:, 2 * g:2 * g + 2, :, :]
                nc.scalar.copy(
                    h_slice, q32g.rearrange("p (j i) c -> p j i c", i=2))
                # fp8 lo: remainder via Pool (q16 - q8h), then ACT cast
                ql16g = qscr.tile([P, 4, C], F16, tag="ql")
                nc.vector.tensor_sub(
                    ql16g, q16g,
                    h_slice.rearrange("p j i c -> p (j i) c"))
                nc.scalar.copy(
                    q8l[:, 2 * g:2 * g + 2, :, :],
                    ql16g.rearrange("p (j i) c -> p j i c", i=2))
                # qT16 via PE transposes
                for t in range(4):
                    b = 4 * g + t
                    tp_ps = psum.tile([P, 8, P, 2], F8, tag="pt",
                                      padded_shape=[P, 8, P, 2])
                    vp = tp_ps.bitcast(F16)   # [P, 8, P, 1] fp16 view
                    for k in range(KC):
                        nc.tensor.transpose(
                            vp[:, k, :, 0], q16g[:, t, k * P:(k + 1) * P],
                            ident16)
                    nc.vector.tensor_copy(
                        qT16[:, :, b * P:(b + 1) * P],
                        vp[:, 0:KC, :, 0])
                # chunk g of any band only needs qT16 columns from group g,
                # so bands 0 and 1 fill during the load loop (PE is idle here)
                e_chunk(e16_0, 0, g, hold=hold0)
                e_chunk(e16_1, 1, g, hold=hold1)
            for bi, eb in ((0, e16_0), (1, e16_1)):
                nc.sync.dma_start(
                    out=scratch[4:, bi, :, :].rearrange("v r c -> r v c"),
                    in_=eb[:, 4 * P:].rearrange("r (v c) -> r v c", c=P),
                )
            e_finish(e16_0, 0)
            e_finish(e16_1, 1)


            # ---- main loop, software-pipelined: emit E(i) then softmax/O(i-1) ----
            def e_phase(i):
                e16 = ework.tile([P, N], F16, tag="e16")
                jmin = i // 4
                if jmin > 0:
                    # lower panel: per-block-transposed read of earlier bands'
                    # scratch blocks (u < 4*jmin)
                    nb = 4 * jmin
                    nc.sync.dma_start_transpose(
                        out=e16[:, :nb * P],
                        in_=scratch[i, :nb, :, :].rearrange(
                            "u r c -> (u r) c"),
                    )
                hold = [None]
                for j in range(jmin, NCH):
                    e_chunk(e16, i, j, jmin, hold=hold)
                if jmin < NCH - 1:
                    # publish right-of-diagonal blocks for future bands
                    v0 = 4 * (jmin + 1)
                    nc.sync.dma_start(
                        out=scratch[v0:, i, :, :].rearrange("v r c -> r v c"),
                        in_=e16[:, v0 * P:].rearrange(
                            "r (v c) -> r v c", c=P),
                    )
                e_finish(e16, i)

            def p_exp(i):
                e16, bias = saved.pop(i)
                # attention weights in fp8, row layout. Emitted before band
                # (i+depth)'s e_phase so ACT's FIFO runs exp ahead of the
                # next band's psum copies.
                t8 = t8p.tile([P, N], F8, tag="t8")
                for h in range(2):
                    nc.scalar.activation(
                        t8[:, h * 2048:(h + 1) * 2048],
                        e16[:, h * 2048:(h + 1) * 2048],
                        mybir.ActivationFunctionType.Exp,
                        bias=bias, scale=-1.0,
                    )
                saved[i] = t8

            def p_phase(i):
                t8 = saved.pop(i)
                # prefetch x rows for the residual add
                xb = xres.tile([P, C], F32, tag="x")
                nc.sync.dma_start(out=xb, in_=x_d[i * P:(i + 1) * P, :])
                # transpose to [v, u] blocks (fp8 step-2 outputs), pipelined
                # one PSUM-bank tile ahead of the O matmuls that consume them
                t8t = t8tp.tile([P, NPAIR, 2, P, 2], F8, tag="t8t")
                o_ps = psum.tile([P, C], F32, tag="o", bufs=1)
                z_ps = psum.tile([P, 1], F32, tag="z", bufs=1)

                def transp_tile(tq):
                    pt = psum.tile([P, 8, P, 2], F8, tag="pt",
                                   padded_shape=[P, 8, P, 2], name="pt")
                    if os.environ.get("DIAG_MEMSET"):
                        # CoreSim-only: the fp8 step-2 transpose leaves odd
                        # bytes uninitialized in the interpreter (HW zeroes
                        # them); memset so the bitcast readback is defined.
                        nc.vector.memset(pt.bitcast(F16), 0.0)
                    for b in range(8):
                        v = 8 * tq + b
                        nc.tensor.transpose(
                            pt[:, b, :, 0], t8[:, v * P:(v + 1) * P], ident8)
                    # late bands have little E-copy work on ACT but are
                    # DVE-bound: split the psum readbacks
                    if i >= 20 and tq % 2 == 1:
                        nc.scalar.copy(
                            t8t[:, 4 * tq:4 * tq + 4, :, :, :].bitcast(F16),
                            pt.bitcast(F16))
                    else:
                        nc.vector.tensor_copy(
                            t8t[:, 4 * tq:4 * tq + 4, :, :, :].bitcast(F16),
                            pt.bitcast(F16),
                        )

                def o_tile(tq):
                    # O = T8 @ (q8h + q8l), Z = T8 @ 1 -- fp8 DoubleRow
                    for j in range(4 * tq, 4 * tq + 4):
                        lhsT = t8t[:, j, :, :, 0]
                        nc.tensor.matmul(o_ps, lhsT, q8h[:, j, :, :],
                                         start=(j == 0), stop=False,
                                         perf_mode=DR)
                        nc.tensor.matmul(o_ps, lhsT, q8l[:, j, :, :],
                                         start=False, stop=(j == NPAIR - 1),
                                         perf_mode=DR)
                        nc.tensor.matmul(z_ps, lhsT, ones8[:, :, 0:1],
                                         start=(j == 0), stop=(j == NPAIR - 1),
                                         perf_mode=DR)

                transp_tile(0)
                for tq in range(4):
                    if tq + 1 < 4:
                        transp_tile(tq + 1)
                    o_tile(tq)

                rz = stats.tile([P, 1], F32, tag="rz")
                nc.vector.reciprocal(rz, z_ps)
                s = stats.tile([P, 1], F32, tag="s")
                nc.vector.tensor_mul(s, rz, alpha_sb)
                o_sb = outp.tile([P, C], F32, tag="o")
                nc.scalar.mul(o_sb, o_ps, mul=s)
                yt = outp.tile([P, C], F32, tag="y")
                nc.vector.tensor_add(yt, o_sb, xb)
                nc.sync.dma_start(out=y_d[i * P:(i + 1) * P, :], in_=yt)

            import os
            nt_run = int(os.environ.get("SIM_BANDS", NT))
            depth = 3 if nt_run > 3 else 1
            for i in range(2, nt_run + depth):
                if i - depth >= 0:
                    p_exp(i - depth)
                if i < nt_run:
                    e_phase(i)
                if i - depth >= 0:
                    p_phase(i - depth)

    _split_matmul_waits(nc)
    return nc


def _split_matmul_waits(nc):
    """Several TRN2 instruction structs (Matmult/Ldweights self-loading path,
    Activation) carry at most ONE sync wait; Tile sometimes emits more. Fix
    by inserting same-engine NoOps immediately before the offender, each
    carrying one surplus wait. A wait moved onto the directly-preceding
    instruction of the same engine is strictly more conservative, so safe."""
    import bass_rust

    LIMITED = {"InstMatmult", "InstLdweights", "InstActivation",
               "InstDmaTransposeAnt", "InstTensorTensor", "InstTensorCopy",
               "InstTensorReduce", "InstReciprocal", "InstTensorScalarPtr",
               "InstTensorScalarAffineSelect", "InstMemset", "InstIota",
               "InstCopyPredicated", "InstTensorScalar", "InstDMACopy",
               "InstDrain", "InstISA"}
    n_nops = 0
    for bb in nc.m.functions[0].blocks:
        insts = list(bb.instructions)
        out = []
        for inst in insts:
            tn = type(inst).__name__
            si = inst.sync_info
            waits = list(si.on_wait) if si else []
            if tn in LIMITED and len(waits) > 1:
                # if directly preceded by this matmul's Ldweights, put the
                # nops before the LDW to keep the LDW+MM pair adjacent
                ins_at = len(out)
                if (tn == "InstMatmult" and out
                        and type(out[-1]).__name__ == "InstLdweights"):
                    ins_at = len(out) - 1
                for w in waits[:-1]:
                    nop = bass_rust.InstNoOp(
                        name=f"I-waitfix-{n_nops}", ins=[], outs=[])
                    nop.engine = inst.engine
                    nop.sync_info = mybir.SyncInfo(on_wait=[w], on_update=[])
                    out.insert(ins_at, nop)
                    ins_at += 1
                    n_nops += 1
                inst.sync_info = mybir.SyncInfo(
                    on_wait=waits[-1:], on_update=list(si.on_update))
            out.append(inst)
        if len(out) != len(insts):
            bb.instructions = out
    return n_nops


def kernel(x, alpha):
    global LAST_RESULTS
    import os
    import time
    # This environment has no NTFF profiling hook (antenv.axon_hooks); a set
    # BASS_TRACE would crash the axon redirect, so force the no-trace path.
    os.environ.setdefault("BASS_NEVER_TRACE", "1")

    x = np.asarray(x, dtype=np.float32)
    alpha = np.asarray(alpha, dtype=np.float32)
    if "nc" not in _CACHE:
        _CACHE["nc"] = _build_bass()
    nc = _CACHE["nc"]

    in_maps = [
        {"x": np.ascontiguousarray(x[b].reshape(N, C)),
         "alpha": alpha.reshape(1, 1)}
        for b in range(B)
    ]
    res = None
    for attempt in range(3):
        try:
            res = run_bass_kernel_spmd(nc, in_maps, list(range(B)))
            break
        except Exception:
            # transient NRT/axon device errors have been observed; retry
            if attempt == 2:
                raise
            time.sleep(5)
    LAST_RESULTS = res
    out = np.stack([res.results[b]["y"].reshape(H, W, C) for b in range(B)])
    return out
